# revision 3
# baseline (speedup 1.0000x reference)
"""KMeansPalettizedConv2d on 8 TRN2 NeuronCores.

Empirical cost model of this axon environment (measured by calibration
variants, see bench_variants.py):
  - straight-line instruction dispatch ~55us/instr (per engine queue);
  - For_i loop bodies replay cheaply (~1.5-2us/matmul) but only while
    the body stays small (~40 instrs/engine); bigger bodies thrash and
    approach straight-line cost;
  - each For_i iteration carries a fixed ~170-200us cost (backedge +
    semaphore-reset block refetch), which grows with the number of
    distinct semaphores the body/preamble touches;
  - cross-engine dependencies are cheap only when the wait is already
    satisfied when checked (deep buffering), otherwise ~0.1-1ms stalls.

Design (per core, data-parallel over batch: 4 images/core):
  One For_i(0, 28) over 8-row output tiles v. All PE-queue APs are
  compile-time constants:
  - a single per-iteration DMA stages the 10-row input window (both
    128-channel halves) from DRAM into a fixed SBUF tile (dynamic DRAM
    src is fine);
  - 36 f32 self-loading matmuls accumulate cout(2) x ci-half(2) x 9
    taps into one fixed [128,2,448] psum tile;
  - one DVE tensor_add drains psum + bias into a big SBUF output
    buffer at a register offset (v*448);
  - iterations 0..15 also run one GpSimd ap_gather + one SBUF->SBUF
    redistribution DMA dequantizing the NEXT rep's weights into the
    other weight buffer (ping-pong per rep);
  - one straight-line DMA per rep stores the output buffer to DRAM.
"""
import numpy as np

import concourse.mybir as mybir
import concourse.tile as tile
from concourse import bacc
from concourse.bass import ds
from concourse.bass_utils import run_bass_kernel_spmd

F32 = mybir.dt.float32
I16 = mybir.dt.int16

N_CORES = 8
N_IMG = 4
HP = 58
NI = 4608
NV = N_IMG * 7
_NC_CACHE = {}


def build_nc(loop_reps=1, skip_dq=False, skip_out=False, dq_mode="full"):
    nc = bacc.Bacc("TRN2", target_bir_lowering=False, debug=False,
                   num_devices=N_CORES)
    # xr[v][ci 128, a 2, 10*58]: 10-row input window for tile v, both halves
    xr = nc.dram_tensor("xr", [NV, 128, 2, 10 * HP], F32,
                        kind="ExternalInput")
    bmap = nc.dram_tensor("bmap", [128, 2, 448], F32, kind="ExternalInput")
    tab = nc.dram_tensor("tab", [128, 256], F32, kind="ExternalInput")
    widx = nc.dram_tensor("widx", [128, NI], I16, kind="ExternalInput")
    out3 = nc.dram_tensor("out3", [128, 2, NV * 448], F32,
                          kind="ExternalOutput")

    with tile.TileContext(nc) as tc:
        with (
            tc.tile_pool(name="const", bufs=1) as constp,
            tc.tile_pool(name="wf", bufs=1) as wfp,
            tc.tile_pool(name="slab", bufs=1) as slabp,
            tc.tile_pool(name="sg", bufs=1) as sgp,
            tc.tile_pool(name="ps", bufs=1, space="PSUM") as psp,
            tc.tile_pool(name="ob", bufs=1) as obp,
        ):
            bmap_sb = constp.tile([128, 2, 448], F32)
            nc.sync.dma_start(bmap_sb[:], bmap[:])
            tab_sb = constp.tile([128, 256], F32)
            nc.sync.dma_start(tab_sb[:], tab[:])
            idx_sb = constp.tile([128, NI], I16)
            nc.sync.dma_start(idx_sb[:], widx[:])

            out_sb = obp.tile([128, 2, NV * 448], F32, tag="out")
            wf = [wfp.tile([128, NI], F32, tag=f"wf{i}", name=f"wf{i}")
                  for i in range(2)]
            slab = [slabp.tile([128, NI], F32, tag=f"sl{i}", name=f"sl{i}")
                    for i in range(2)]
            wfL = wfp.tile([128, NI], F32, tag="wfL", name="wfL")
            # stage tile: [ci 128, a 2, 10 rows, 58]
            sgt = sgp.tile([128, 2, 10, HP], F32, tag="sg")

            def gather_piece(r):
                s = slab[r % 2]
                nc.gpsimd.ap_gather(
                    s[:], tab_sb[:],
                    idx_sb[:, r * (NI // 16):(r + 1) * (NI // 16)],
                    channels=128, num_elems=256, d=1, num_idxs=NI)
                return s

            def dequant_straight(dst):
                # software-pipelined: keep 2 gathers in flight ahead of
                # each redistribution DMA so no wait ever blocks
                gather_piece(0)
                for r in range(16):
                    if r + 1 < 16:
                        gather_piece(r + 1)
                    s = slab[r % 2]
                    nc.scalar.dma_start(dst[r::16, :], s[r::16, :])

            def conv(wcur, wnext):
                nc.sync.dma_start(wfL[:], wcur[:])
                with tc.For_i(0, NV, back_edge_label="convbe",
                              hint_engines=tuple(mybir.ALL_ENGINES)) as u:
                    tc.mark_branch_hint_location(
                        "convbe", engines=mybir.ALL_ENGINES)
                    nc.sync.dma_start(sgt[:], xr[ds(u, 1)])
                    ps = psp.tile([128, 2, 512], F32, tag="ps", name="ps")
                    for b in range(2):
                        for a in range(2):
                            for kk in range(9):
                                ky, kx = kk // 3, kk % 3
                                t_idx = b * 18 + a * 9 + kk
                                nc.tensor.matmul(
                                    ps[:, b, 0:448],
                                    wfL[:, t_idx * 128:(t_idx + 1) * 128],
                                    sgt[:, a, ky:ky + 8, kx:kx + 56],
                                    start=(a == 0 and kk == 0),
                                    stop=(a == 1 and kk == 8))
                    nc.vector.tensor_add(
                        out_sb[:, :, ds(u * 448, 448)],
                        ps[:, :, 0:448], bmap_sb[:])

            def conv_dq(wcur, wnext):
                # same loop, plus one gather+dma pair per early iteration
                # dequantizing wnext for the following rep
                with tc.For_i(0, NV) as u:
                    nc.sync.dma_start(sgt[:], xr[ds(u, 1)])
                    ps = psp.tile([128, 2, 512], F32, tag="ps", name="ps")
                    for b in range(2):
                        for a in range(2):
                            for kk in range(9):
                                ky, kx = kk // 3, kk % 3
                                t_idx = b * 18 + a * 9 + kk
                                nc.tensor.matmul(
                                    ps[:, b, 0:448],
                                    wfL[:, t_idx * 128:(t_idx + 1) * 128],
                                    sgt[:, a, ky:ky + 8, kx:kx + 56],
                                    start=(a == 0 and kk == 0),
                                    stop=(a == 1 and kk == 8))
                    nc.vector.tensor_add(
                        out_sb[:, :, ds(u * 448, 448)],
                        ps[:, :, 0:448], bmap_sb[:])
                # gathers as a second short loop would re-pay loop floor;
                # instead run them straight-line AFTER the conv loop - they
                # overlap the NEXT rep's conv via buffering... no: barriers.
                # Keep them inside the iteration space instead:

            # NOTE: gathers cannot live inside For_i without blowing the
            # body for iterations 0..15 only (python emits per-iteration
            # different bodies is impossible in one For_i). Instead:
            # dequant for rep r+1 runs straight-line between rep r's conv
            # loop and rep r+1's conv loop (32 instrs ~ 1.8ms... measured).
            for r in range(loop_reps):
                if r == 0:
                    dequant_straight(wf[0])
                wi = 0 if skip_dq else r % 2
                conv(wf[wi], None)
                if r + 1 < loop_reps and not skip_dq:
                    dequant_straight(wf[(r + 1) % 2])
                if not skip_out or r == loop_reps - 1:
                    nc.scalar.dma_start(out3[:], out_sb[:])
    nc.finalize()
    return nc


def prep_inputs(input, weight_idx, lookup_table, bias):
    input = np.asarray(input)
    weight_idx = np.asarray(weight_idx)
    lookup_table = np.asarray(lookup_table, dtype=np.float32)
    bias = np.asarray(bias, dtype=np.float32)

    xp = np.zeros((32, 256, HP, HP), np.float32)
    xp[:, :, 1:57, 1:57] = input
    xp4 = xp.reshape(32, 2, 128, HP, HP)

    bmap = np.ascontiguousarray(
        np.broadcast_to(bias.reshape(2, 128).T[:, :, None], (128, 2, 448))
        .astype(np.float32))

    A = weight_idx.reshape(2, 128, 2, 8, 16, 9)      # [b, co, a, g, r, kk]
    L = A.transpose(4, 3, 0, 2, 5, 1).reshape(16, 8, NI)  # [r, g, j]
    widx = (L.reshape(16, 8, NI // 16, 16)
             .transpose(0, 1, 3, 2)
             .reshape(16, 128, NI // 16)
             .transpose(1, 0, 2)
             .reshape(128, NI)
             .astype(np.int16))

    com = {
        "bmap": bmap,
        "widx": np.ascontiguousarray(widx),
        "tab": np.broadcast_to(lookup_table, (128, 256)).copy(),
    }
    maps = []
    for c in range(N_CORES):
        # xr[v][ci, a, 10*58]
        xr = np.empty((NV, 128, 2, 10 * HP), np.float32)
        for i in range(N_IMG):
            img = xp4[c * N_IMG + i]            # [a 2, ci 128, 58, 58]
            for rt in range(7):
                win = img[:, :, rt * 8:rt * 8 + 10, :]   # [2,128,10,58]
                xr[i * 7 + rt] = win.transpose(1, 0, 2, 3).reshape(
                    128, 2, 10 * HP)
        maps.append({"xr": xr, **com})
    return maps


def run(in_maps, loop_reps=1, cores=None, skip_dq=False, skip_out=False,
        dq_mode="full"):
    key = (loop_reps, skip_dq, skip_out, dq_mode)
    if key not in _NC_CACHE:
        _NC_CACHE[key] = build_nc(loop_reps, skip_dq, skip_out, dq_mode)
    if cores is None:
        cores = list(range(N_CORES))
    return run_bass_kernel_spmd(_NC_CACHE[key], in_maps[:len(cores)],
                                core_ids=cores)


def kernel(input, weight_idx, lookup_table, bias):
    in_maps = prep_inputs(input, weight_idx, lookup_table, bias)
    res = run(in_maps)
    outs = [res.results[c]["out3"] for c in range(N_CORES)]
    full = np.stack(outs, axis=0)                # [8, 128, 2, 28*448]
    full = full.reshape(N_CORES, 128, 2, N_IMG, 3136)
    full = full.transpose(0, 3, 2, 1, 4)         # [8, 4, 2, 128, 3136]
    return np.ascontiguousarray(full).reshape(32, 256, 56, 56)


# revision 4
# speedup vs baseline: 1.0064x; 1.0064x over previous
"""KMeansPalettizedConv2d on 8 TRN2 NeuronCores.

Empirical cost model of this axon environment (measured by calibration
variants, see bench_variants.py):
  - straight-line instruction dispatch ~55us/instr (per engine queue);
  - For_i loop bodies replay cheaply (~1.5-2us/matmul) but only while
    the body stays small (~40 instrs/engine); bigger bodies thrash and
    approach straight-line cost;
  - each For_i iteration carries a fixed ~170-200us cost (backedge +
    semaphore-reset block refetch), which grows with the number of
    distinct semaphores the body/preamble touches;
  - cross-engine dependencies are cheap only when the wait is already
    satisfied when checked (deep buffering), otherwise ~0.1-1ms stalls.

Design (per core, data-parallel over batch: 4 images/core):
  One For_i(0, 28) over 8-row output tiles v. All PE-queue APs are
  compile-time constants:
  - a single per-iteration DMA stages the 10-row input window (both
    128-channel halves) from DRAM into a fixed SBUF tile (dynamic DRAM
    src is fine);
  - 36 f32 self-loading matmuls accumulate cout(2) x ci-half(2) x 9
    taps into one fixed [128,2,448] psum tile;
  - one DVE tensor_add drains psum + bias into a big SBUF output
    buffer at a register offset (v*448);
  - per rep, 16 GpSimd ap_gather pieces + 16 partition-strided (r::16)
    redistribution DMAs dequantize the weights (the gather ISA shares
    one index stream per 16-partition group, forcing 16 pieces); the
    next rep's weights are dequanted into the other ping-pong buffer,
    and one big copy consolidates them into wfL (single DMA queue, so
    the loop body re-checks one semaphore instead of eight);
  - one straight-line DMA per rep stores the output buffer to DRAM.
"""
import numpy as np

import concourse.mybir as mybir
import concourse.tile as tile
from concourse import bacc
from concourse.bass import ds
from concourse.bass_utils import run_bass_kernel_spmd

F32 = mybir.dt.float32
I16 = mybir.dt.int16

N_CORES = 8
N_IMG = 4
HP = 58
NI = 4608
NV = N_IMG * 7
_NC_CACHE = {}


def build_nc(loop_reps=1, skip_dq=False, skip_out=False, dq_mode="full"):
    nc = bacc.Bacc("TRN2", target_bir_lowering=False, debug=False,
                   num_devices=N_CORES)
    # xr[v][ci 128, a 2, 10*58]: 10-row input window for tile v, both halves
    xr = nc.dram_tensor("xr", [NV, 128, 2, 10 * HP], F32,
                        kind="ExternalInput")
    bmap = nc.dram_tensor("bmap", [128, 2, 448], F32, kind="ExternalInput")
    tab = nc.dram_tensor("tab", [128, 256], F32, kind="ExternalInput")
    widx = nc.dram_tensor("widx", [128, NI], I16, kind="ExternalInput")
    out3 = nc.dram_tensor("out3", [128, 2, NV * 448], F32,
                          kind="ExternalOutput")

    with tile.TileContext(nc) as tc:
        with (
            tc.tile_pool(name="const", bufs=1) as constp,
            tc.tile_pool(name="wf", bufs=1) as wfp,
            tc.tile_pool(name="slab", bufs=1) as slabp,
            tc.tile_pool(name="sg", bufs=1) as sgp,
            tc.tile_pool(name="ps", bufs=1, space="PSUM") as psp,
            tc.tile_pool(name="ob", bufs=1) as obp,
        ):
            bmap_sb = constp.tile([128, 2, 448], F32)
            nc.sync.dma_start(bmap_sb[:], bmap[:])
            tab_sb = constp.tile([128, 256], F32)
            nc.sync.dma_start(tab_sb[:], tab[:])
            idx_sb = constp.tile([128, NI], I16)
            nc.sync.dma_start(idx_sb[:], widx[:])

            out_sb = obp.tile([128, 2, NV * 448], F32, tag="out")
            wf = [wfp.tile([128, NI], F32, tag=f"wf{i}", name=f"wf{i}")
                  for i in range(2)]
            slab = [slabp.tile([128, NI], F32, tag=f"sl{i}", name=f"sl{i}")
                    for i in range(2)]
            wfL = wfp.tile([128, NI], F32, tag="wfL", name="wfL")
            # stage tile: [ci 128, a 2, 10 rows, 58]
            sgt = sgp.tile([128, 2, 10, HP], F32, tag="sg")

            def gather_piece(r):
                s = slab[r % 2]
                nc.gpsimd.ap_gather(
                    s[:], tab_sb[:],
                    idx_sb[:, r * (NI // 16):(r + 1) * (NI // 16)],
                    channels=128, num_elems=256, d=1, num_idxs=NI)
                return s

            def dequant_straight(dst):
                # software-pipelined: keep one gather in flight ahead of
                # each redistribution DMA (2-slab ping-pong)
                gather_piece(0)
                for r in range(16):
                    if r + 1 < 16:
                        gather_piece(r + 1)
                    s = slab[r % 2]
                    nc.scalar.dma_start(dst[r::16, :], s[r::16, :])

            def conv(wcur, wnext):
                nc.sync.dma_start(wfL[:], wcur[:])
                with tc.For_i(0, NV, back_edge_label="convbe",
                              hint_engines=tuple(mybir.ALL_ENGINES)) as u:
                    tc.mark_branch_hint_location(
                        "convbe", engines=mybir.ALL_ENGINES)
                    nc.sync.dma_start(sgt[:], xr[ds(u, 1)])
                    ps = psp.tile([128, 2, 512], F32, tag="ps", name="ps")
                    for b in range(2):
                        for a in range(2):
                            for kk in range(9):
                                ky, kx = kk // 3, kk % 3
                                t_idx = b * 18 + a * 9 + kk
                                nc.tensor.matmul(
                                    ps[:, b, 0:448],
                                    wfL[:, t_idx * 128:(t_idx + 1) * 128],
                                    sgt[:, a, ky:ky + 8, kx:kx + 56],
                                    start=(a == 0 and kk == 0),
                                    stop=(a == 1 and kk == 8))
                    nc.vector.tensor_add(
                        out_sb[:, :, ds(u * 448, 448)],
                        ps[:, :, 0:448], bmap_sb[:])

            for r in range(loop_reps):
                if r == 0:
                    dequant_straight(wf[0])
                wi = 0 if skip_dq else r % 2
                conv(wf[wi], None)
                if r + 1 < loop_reps and not skip_dq:
                    dequant_straight(wf[(r + 1) % 2])
                if not skip_out or r == loop_reps - 1:
                    nc.scalar.dma_start(out3[:], out_sb[:])
    nc.finalize()
    return nc


def prep_inputs(input, weight_idx, lookup_table, bias):
    input = np.asarray(input)
    weight_idx = np.asarray(weight_idx)
    lookup_table = np.asarray(lookup_table, dtype=np.float32)
    bias = np.asarray(bias, dtype=np.float32)

    xp = np.zeros((32, 256, HP, HP), np.float32)
    xp[:, :, 1:57, 1:57] = input
    xp4 = xp.reshape(32, 2, 128, HP, HP)

    bmap = np.ascontiguousarray(
        np.broadcast_to(bias.reshape(2, 128).T[:, :, None], (128, 2, 448))
        .astype(np.float32))

    A = weight_idx.reshape(2, 128, 2, 8, 16, 9)      # [b, co, a, g, r, kk]
    L = A.transpose(4, 3, 0, 2, 5, 1).reshape(16, 8, NI)  # [r, g, j]
    widx = (L.reshape(16, 8, NI // 16, 16)
             .transpose(0, 1, 3, 2)
             .reshape(16, 128, NI // 16)
             .transpose(1, 0, 2)
             .reshape(128, NI)
             .astype(np.int16))

    com = {
        "bmap": bmap,
        "widx": np.ascontiguousarray(widx),
        "tab": np.broadcast_to(lookup_table, (128, 256)).copy(),
    }
    maps = []
    for c in range(N_CORES):
        # xr[v][ci, a, 10*58]
        xr = np.empty((NV, 128, 2, 10 * HP), np.float32)
        for i in range(N_IMG):
            img = xp4[c * N_IMG + i]            # [a 2, ci 128, 58, 58]
            for rt in range(7):
                win = img[:, :, rt * 8:rt * 8 + 10, :]   # [2,128,10,58]
                xr[i * 7 + rt] = win.transpose(1, 0, 2, 3).reshape(
                    128, 2, 10 * HP)
        maps.append({"xr": xr, **com})
    return maps


def run(in_maps, loop_reps=1, cores=None, skip_dq=False, skip_out=False,
        dq_mode="full"):
    key = (loop_reps, skip_dq, skip_out, dq_mode)
    if key not in _NC_CACHE:
        _NC_CACHE[key] = build_nc(loop_reps, skip_dq, skip_out, dq_mode)
    if cores is None:
        cores = list(range(N_CORES))
    return run_bass_kernel_spmd(_NC_CACHE[key], in_maps[:len(cores)],
                                core_ids=cores)


def kernel(input, weight_idx, lookup_table, bias):
    in_maps = prep_inputs(input, weight_idx, lookup_table, bias)
    res = run(in_maps)
    outs = [res.results[c]["out3"] for c in range(N_CORES)]
    full = np.stack(outs, axis=0)                # [8, 128, 2, 28*448]
    full = full.reshape(N_CORES, 128, 2, N_IMG, 3136)
    full = full.transpose(0, 3, 2, 1, 4)         # [8, 4, 2, 128, 3136]
    return np.ascontiguousarray(full).reshape(32, 256, 56, 56)
